# revision 16
# baseline (speedup 1.0000x reference)
"""Trainium2 Bass kernel for nn_FPModule (knn-interpolate + 2-layer MLP).

Strategy (per sharding hint): shard the 16 clouds over 8 NeuronCores, 2
clouds/core; kNN + interpolation + MLP fully local per cloud; MLP weights
replicated.

Per-core pipeline (per cloud, nc=1024 coarse, nf=4096 fine):
 1. PE: s = -d2[fine, coarse] via a K=13 augmented fp16 hi/lo split matmul
    (centroid-shifted coords; fp32-accurate: |err| ~3e-7, validated 0
    neighbor flips vs the fp32 reference formula).
 2. DVE: max8 + max_index per 128-row chunk -> top-3 values (= -d2) and
    coarse indices.
 3. Inverse-distance weights from the exact top-3 values (tiny DVE ops).
 4. dma_gather: fetch the 3 neighbor feature rows (fp16) per fine point.
 5. DVE: weighted MAC -> y[fine, 256] fp16; PE-transpose to y.T.
 6. PE: MLP in transposed orientation (weights stationary, fine moving):
    h.T = relu(W1.T @ [y;xs].T + b1), out.T = relu(W2.T @ h.T + b2).
 7. DMA out.T back; host reassembles [65536, 256] fp32.
"""

import os
import sys
import numpy as np

sys.path.insert(0, "/opt/trn_rl_repo")

import concourse.bass as bass
import concourse.bacc as bacc
import concourse.mybir as mybir
from concourse.tile import TileContext
from concourse import bass_utils

FP16 = mybir.dt.float16
FP32 = mybir.dt.float32
U16 = mybir.dt.uint16
U32 = mybir.dt.uint32
I16 = mybir.dt.int16

B, NCO, NF = 16, 1024, 4096
CIN, CSK, CMID, COUT = 256, 128, 256, 256
CORES, CPC = 8, 2            # cores, clouds per core
CH = NF // 128               # 32 chunks of 128 fine points per cloud
KAUG = 13                    # augmented contraction rows for s = -d2
K = 3
J = 4                        # re-rank candidate count
ROWE = CIN + 8               # xaug row elements (256 feat + 12B fp32 pos + pad)

TENSOR_SPECS = [
    # name, shape, dtype, kind
    ("qa",   [CPC, KAUG, NF],       FP16, "ExternalInput"),
    ("pa",   [CPC, KAUG, NCO],      FP16, "ExternalInput"),
    ("xg",   [CPC * NCO, CIN + 8],  FP16, "ExternalInput"),
    ("qpos", [CPC, 128, 512],       FP32, "ExternalInput"),
    ("xst",  [CPC, CSK, NF],        FP16, "ExternalInput"),
    ("w1",   [128, 3 * 2 * 128],    FP16, "ExternalInput"),
    ("w2",   [128, 2 * 2 * 128],    FP16, "ExternalInput"),
    ("b1",   [128, 2],              FP32, "ExternalInput"),
    ("b2",   [128, 2],              FP32, "ExternalInput"),
    ("ident", [128, 128],           FP16, "ExternalInput"),
    ("outT", [CPC, 2, 128, NF],     FP32, "ExternalOutput"),
]


def build_kernel(tc, outs, ins):
    """Emit the per-core program. outs/ins: dicts name -> bass.AP."""
    from contextlib import ExitStack
    nc = tc.nc
    io = dict(ins)
    io.update(outs)

    with ExitStack() as ctx:
        const = ctx.enter_context(tc.tile_pool(name="const", bufs=1))
        spool = ctx.enter_context(tc.tile_pool(name="spsum", bufs=2, space="PSUM"))
        ypsum = ctx.enter_context(tc.tile_pool(name="ypsum", bufs=1, space="PSUM"))
        mpsum = ctx.enter_context(tc.tile_pool(name="mpsum", bufs=2, space="PSUM"))
        scan = ctx.enter_context(tc.tile_pool(name="scan", bufs=1))
        gpool = ctx.enter_context(tc.tile_pool(name="gath", bufs=3))
        ypool = ctx.enter_context(tc.tile_pool(name="ymac", bufs=3))
        tpool = ctx.enter_context(tc.tile_pool(name="tmp", bufs=3))
        ytpool = ctx.enter_context(tc.tile_pool(name="ytp", bufs=1))
        hpool = ctx.enter_context(tc.tile_pool(name="hp", bufs=1))
        opool = ctx.enter_context(tc.tile_pool(name="op", bufs=4))

        # ---- constant loads (per core) ----
        def load_const(name, shape, dtype, src_ap, tag):
            t = const.tile(shape, dtype, tag=tag)
            nc.sync.dma_start(t[:], src_ap)
            return t

        qa_sb, pa_sb, xst_sb = [], [], []
        for cl in range(CPC):
            qa_sb.append(load_const("qa", [KAUG, NF], FP16, io["qa"][cl], f"qa{cl}"))
            pa_sb.append(load_const("pa", [KAUG, NCO], FP16, io["pa"][cl], f"pa{cl}"))
            xst_sb.append(load_const("xst", [CSK, NF], FP16, io["xst"][cl], f"xst{cl}"))
        w1_sb = load_const("w1", [128, 3 * 2 * 128], FP16, io["w1"], "w1")
        w2_sb = load_const("w2", [128, 2 * 2 * 128], FP16, io["w2"], "w2")
        b1_sb = load_const("b1", [128, 2], FP32, io["b1"], "b1")
        b2_sb = load_const("b2", [128, 2], FP32, io["b2"], "b2")
        id_sb = load_const("ident", [128, 128], FP16, io["ident"], "ident")

        def w1t(k, mt):
            return w1_sb[:, (k * 2 + mt) * 128:(k * 2 + mt + 1) * 128]

        def w2t(k, mt):
            return w2_sb[:, (k * 2 + mt) * 128:(k * 2 + mt + 1) * 128]

        for cl in range(CPC):
            VALS = scan.tile([128, CH * 8], FP32, tag=f"vals{cl}")
            IDX = scan.tile([128, CH * 8], U32, tag=f"idx{cl}")
            D2N = scan.tile([128, CH * 8], FP32, tag=f"d2n{cl}")
            M8 = scan.tile([128, CH * 8], FP32, tag=f"m8{cl}")
            WT = scan.tile([128, CH * J], FP32, tag=f"wt{cl}")
            SEL = scan.tile([128, CH * J], FP32, tag=f"sel{cl}")
            RC = scan.tile([128, CH * J], FP32, tag=f"rc{cl}")
            WS = scan.tile([128, CH], FP32, tag=f"ws{cl}")
            RS = scan.tile([128, CH], FP32, tag=f"rs{cl}")
            qp_sb = scan.tile([128, 512], FP32, tag=f"qp{cl}")
            nc.sync.dma_start(qp_sb[:], io["qpos"][cl])

            d2n3 = D2N[:].rearrange("p (c j) -> p c j", j=8)
            # pad candidate slots J..8 once so max8 never selects them
            nc.vector.memset(d2n3[:, :, J:8], -1e30)

            # ---- phase A: s = -d2 on PE; approx top-8 scan on DVE ----
            for c in range(CH):
                sp = spool.tile([128, NCO], FP32, tag="s")
                for h in range(2):
                    nc.tensor.matmul(
                        sp[:, h * 512:(h + 1) * 512],
                        lhsT=qa_sb[cl][:, c * 128:(c + 1) * 128],
                        rhs=pa_sb[cl][:, h * 512:(h + 1) * 512],
                        start=True, stop=True,
                    )
                nc.vector.max(VALS[:, c * 8:(c + 1) * 8], sp[:])
                nc.vector.max_index(IDX[:, c * 8:(c + 1) * 8],
                                    VALS[:, c * 8:(c + 1) * 8], sp[:])

            xg_flat = io["xg"]
            ypq = [None, None]
            yt = [ytpool.tile([128, NF], FP16, tag=f"yt{cl}{mt}", name=f"yt{cl}_{mt}")
                  for mt in range(2)]
            for grp in range(4):
                gts = []
                # ---- gather top-J candidates (features + fp32 positions) ----
                for ci in range(8):
                    c = grp * 8 + ci
                    G = gpool.tile([128, J * ROWE], FP16, tag="G", name="G", bufs=9)
                    for k in range(J):
                        nc.gpsimd.indirect_dma_start(
                            out=G[:, k * ROWE:(k + 1) * ROWE], out_offset=None,
                            in_=xg_flat,
                            in_offset=bass.IndirectOffsetOnAxis(
                                ap=IDX[:, c * 8 + k:c * 8 + k + 1], axis=0),
                            element_offset=cl * NCO * ROWE,
                        )
                    gts.append(G)
                    # exact reference-formula d2 for the J candidates
                    gv = G[:].rearrange("p (k e) -> p k e", e=ROWE)
                    posv = gv[:, :, CIN:CIN + 6].bitcast(FP32)  # [128, J, 3]
                    qp3 = qp_sb[:, c * 16:(c + 1) * 16].rearrange(
                        "p (k d) -> p k d", d=4)[:, :, 0:3]
                    dif = tpool.tile([128, J * 3], FP32, tag="dif", name="dif")
                    d3v = dif[:].rearrange("p (k d) -> p k d", d=3)
                    nc.vector.tensor_tensor(d3v, posv, qp3,
                                            op=mybir.AluOpType.subtract)
                    sq = tpool.tile([128, J * 3], FP32, tag="sq", name="sq")
                    s3v = sq[:].rearrange("p (k d) -> p k d", d=3)
                    nc.vector.tensor_tensor(s3v, d3v, d3v, op=mybir.AluOpType.mult)
                    nc.vector.tensor_reduce(d2n3[:, c, 0:J], s3v,
                                            axis=mybir.AxisListType.X,
                                            op=mybir.AluOpType.add, negate=True)
                    nc.vector.max(M8[:, c * 8:(c + 1) * 8],
                                  D2N[:, c * 8:(c + 1) * 8])

                # ---- batched exact weights for the group ----
                gs = slice(grp * 8 * J, (grp + 1) * 8 * J)     # J-wide cols
                gs8 = slice(grp * 8 * 8, (grp + 1) * 8 * 8)    # 8-wide cols
                gsc = slice(grp * 8, (grp + 1) * 8)            # per-chunk cols
                m83 = M8[:, gs8].rearrange("p (c j) -> p c j", j=8)
                dn = D2N[:, gs8].rearrange("p (c j) -> p c j", j=8)[:, :, 0:J]
                sel3 = SEL[:, gs].rearrange("p (c j) -> p c j", j=J)
                nc.vector.tensor_tensor(sel3, dn,
                                        m83[:, :, 2:3].to_broadcast([128, 8, J]),
                                        op=mybir.AluOpType.is_ge)
                rc3 = RC[:, gs].rearrange("p (c j) -> p c j", j=J)
                nc.vector.tensor_scalar(rc3, dn, -1.0, 1e-16,
                                        op0=mybir.AluOpType.mult,
                                        op1=mybir.AluOpType.max)
                nc.vector.reciprocal(RC[:, gs], RC[:, gs])
                nc.vector.tensor_mul(RC[:, gs], RC[:, gs], SEL[:, gs])
                nc.vector.tensor_reduce(WS[:, gsc], rc3,
                                        axis=mybir.AxisListType.X,
                                        op=mybir.AluOpType.add)
                nc.vector.reciprocal(RS[:, gsc], WS[:, gsc])
                wt3 = WT[:, gs].rearrange("p (c j) -> p c j", j=J)
                rs3 = RS[:, gsc].rearrange("p (c o) -> p c o", o=1)
                nc.vector.tensor_tensor(wt3, rc3,
                                        rs3.to_broadcast([128, 8, J]),
                                        op=mybir.AluOpType.mult)

                # ---- weighted MAC + PE transpose of y ----
                for ci in range(8):
                    c = grp * 8 + ci
                    G = gts[ci]

                    def feat(k):
                        return G[:, k * ROWE:k * ROWE + CIN]

                    t1 = tpool.tile([128, CIN], FP16, tag="t1")
                    t2 = tpool.tile([128, CIN], FP16, tag="t2")
                    t3 = tpool.tile([128, CIN], FP16, tag="t3")
                    y_c = ypool.tile([128, CIN], FP16, tag="yc")
                    nc.vector.tensor_scalar(t1[:], feat(0),
                                            WT[:, c * J:c * J + 1],
                                            None, op0=mybir.AluOpType.mult)
                    nc.vector.affine_then_add(t2[:], feat(1), t1[:],
                                              scale=WT[:, c * J + 1:c * J + 2],
                                              bias=0.0)
                    nc.vector.affine_then_add(t3[:], feat(2), t2[:],
                                              scale=WT[:, c * J + 2:c * J + 3],
                                              bias=0.0)
                    nc.vector.affine_then_add(y_c[:], feat(3), t3[:],
                                              scale=WT[:, c * J + 3:c * J + 4],
                                              bias=0.0)
                    q = c % 4
                    for mt in range(2):
                        if q == 0:
                            ypq[mt] = ypsum.tile([128, 512], FP16, tag=f"yp{mt}", name=f"yp{mt}")
                        nc.tensor.transpose(ypq[mt][:, q * 128:(q + 1) * 128],
                                            y_c[:, mt * 128:(mt + 1) * 128], id_sb[:])
                    if q == 3:
                        qq = c // 4
                        for mt in range(2):
                            nc.vector.tensor_copy(yt[mt][:, qq * 512:(qq + 1) * 512],
                                                  ypq[mt][:])

            # ---- phase F: MLP (transposed orientation) ----
            ht = [hpool.tile([128, NF], FP16, tag=f"ht{cl}{mt}", name=f"ht{cl}_{mt}")
                  for mt in range(2)]
            rhs1 = [yt[0], yt[1], xst_sb[cl]]
            for mt in range(2):
                for nt in range(8):
                    mp = mpsum.tile([128, 512], FP32, tag="mp")
                    for k in range(3):
                        nc.tensor.matmul(mp[:], lhsT=w1t(k, mt),
                                         rhs=rhs1[k][:, nt * 512:(nt + 1) * 512],
                                         start=(k == 0), stop=(k == 2))
                    nc.scalar.activation(ht[mt][:, nt * 512:(nt + 1) * 512], mp[:],
                                         mybir.ActivationFunctionType.Relu,
                                         bias=b1_sb[:, mt:mt + 1], scale=1.0)
            for mt in range(2):
                for nt in range(8):
                    mp = mpsum.tile([128, 512], FP32, tag="mp")
                    for k in range(2):
                        nc.tensor.matmul(mp[:], lhsT=w2t(k, mt),
                                         rhs=ht[k][:, nt * 512:(nt + 1) * 512],
                                         start=(k == 0), stop=(k == 1))
                    o = opool.tile([128, 512], FP32, tag="o")
                    nc.vector.tensor_scalar(o[:], mp[:], b2_sb[:, mt:mt + 1], 0.0,
                                            op0=mybir.AluOpType.add,
                                            op1=mybir.AluOpType.max)
                    nc.sync.dma_start(io["outT"][cl, mt][:, nt * 512:(nt + 1) * 512],
                                      o[:])


def host_prep(inputs):
    """Full inputs -> list of 8 per-core in_maps (numpy)."""
    x = np.asarray(inputs["x"], np.float32)
    pos = np.asarray(inputs["pos"], np.float64)
    x_skip = np.asarray(inputs["x_skip"], np.float32)
    pos_skip = np.asarray(inputs["pos_skip"], np.float64)
    W1 = np.asarray(inputs["W1"], np.float32)
    b1 = np.asarray(inputs["b1"], np.float32)
    W2 = np.asarray(inputs["W2"], np.float32)
    b2 = np.asarray(inputs["b2"], np.float32)

    ident = np.eye(128, dtype=np.float16)
    # w[p, (k*2+mt)*128+f] = W[k*128+p, mt*128+f]
    w1r = np.ascontiguousarray(
        W1.astype(np.float16).reshape(3, 128, 2, 128)
        .transpose(1, 0, 2, 3).reshape(128, 3 * 2 * 128))
    w2r = np.ascontiguousarray(
        W2.astype(np.float16).reshape(2, 128, 2, 128)
        .transpose(1, 0, 2, 3).reshape(128, 2 * 2 * 128))
    b1r = np.ascontiguousarray(b1.reshape(2, 128).T)
    b2r = np.ascontiguousarray(b2.reshape(2, 128).T)

    in_maps = []
    for core in range(CORES):
        qa = np.zeros((CPC, KAUG, NF), np.float16)
        pa = np.zeros((CPC, KAUG, NCO), np.float16)
        xg = np.zeros((CPC * NCO, CIN + 8), np.float16)
        qpos = np.zeros((CPC, 128, 512), np.float32)
        xst = np.zeros((CPC, CSK, NF), np.float16)
        for cl in range(CPC):
            cloud = core * CPC + cl
            pf = pos[cloud * NCO:(cloud + 1) * NCO]
            qf = pos_skip[cloud * NF:(cloud + 1) * NF]
            xf = x[cloud * NCO:(cloud + 1) * NCO]
            xsf = x_skip[cloud * NF:(cloud + 1) * NF]

            mu = pf.mean(0)
            qp = qf - mu
            pp = pf - mu
            qh = qp.astype(np.float16)
            ql = (qp - qh.astype(np.float64)).astype(np.float16)
            ph = pp.astype(np.float16)
            pl = (pp - ph.astype(np.float64)).astype(np.float16)
            qs = qh.astype(np.float64) + ql.astype(np.float64)
            ps = ph.astype(np.float64) + pl.astype(np.float64)
            nq2 = -(qs * qs).sum(1)
            np2 = -(ps * ps).sum(1)
            nq2h = nq2.astype(np.float16)
            nq2l = (nq2 - nq2h.astype(np.float64)).astype(np.float16)
            np2h = np2.astype(np.float16)
            np2l = (np2 - np2h.astype(np.float64)).astype(np.float16)
            one_f = np.ones(NF, np.float16)
            one_c = np.ones(NCO, np.float16)
            qa[cl] = np.stack([
                2 * qh[:, 0], 2 * qh[:, 1], 2 * qh[:, 2],
                2 * qh[:, 0], 2 * qh[:, 1], 2 * qh[:, 2],
                2 * ql[:, 0], 2 * ql[:, 1], 2 * ql[:, 2],
                nq2h, nq2l, one_f, one_f])
            pa[cl] = np.stack([
                ph[:, 0], ph[:, 1], ph[:, 2],
                pl[:, 0], pl[:, 1], pl[:, 2],
                ph[:, 0], ph[:, 1], ph[:, 2],
                one_c, one_c, np2h, np2l])
            row = xg[cl * NCO:(cl + 1) * NCO]
            row[:, :CIN] = xf.astype(np.float16)
            # embed raw fp32 coarse positions in the row tail (bytes 512..524)
            row[:, CIN:CIN + 6] = pf.astype(np.float32).view(np.float16)
            # qpos[cl, p, c*16 + k*4 + d] = fine-point coord (fp32, replicated J times)
            qf32 = qf.astype(np.float32).reshape(CH, 128, 3)
            qp4 = np.zeros((CH, 128, J, 4), np.float32)
            qp4[:, :, :, 0:3] = qf32[:, :, None, :]
            qpos[cl] = qp4.transpose(1, 0, 2, 3).reshape(128, CH * 16)
            xst[cl] = np.ascontiguousarray(xsf.astype(np.float16).T)
        in_maps.append({
            "qa": qa, "pa": pa, "xg": xg, "qpos": qpos, "xst": xst,
            "w1": w1r, "w2": w2r, "b1": b1r, "b2": b2r, "ident": ident,
        })
    return in_maps


def host_post(results, inputs):
    out = np.empty((B * NF, COUT), np.float32)
    for core in range(CORES):
        o = results[core]["outT"]  # [CPC, 2, 128, NF]
        for cl in range(CPC):
            cloud = core * CPC + cl
            blk = o[cl].reshape(COUT, NF)  # [256, 4096]
            out[cloud * NF:(cloud + 1) * NF] = blk.T
    pos_skip = np.asarray(inputs["pos_skip"])
    batch_skip = np.asarray(inputs["batch_skip"])
    return (out, pos_skip, batch_skip)


def make_program():
    nc = bacc.Bacc("TRN2", target_bir_lowering=False, debug=False,
                   enable_asserts=False, num_devices=CORES)
    ins, outs = {}, {}
    for name, shape, dt, kind in TENSOR_SPECS:
        ap = nc.dram_tensor(name, shape, dt, kind=kind).ap()
        (outs if kind == "ExternalOutput" else ins)[name] = ap
    with TileContext(nc) as tc:
        build_kernel(tc, outs, ins)
    nc.compile()
    return nc


_PROGRAM_CACHE = {}


def kernel(**inputs):
    in_maps = host_prep(inputs)
    if "nc" not in _PROGRAM_CACHE:
        _PROGRAM_CACHE["nc"] = make_program()
    nc = _PROGRAM_CACHE["nc"]
    res = bass_utils.run_bass_kernel_spmd(
        nc, in_maps, core_ids=list(range(CORES)),
        trace=bool(int(os.environ.get("KERNEL_TRACE", "0"))),
    )
    out = host_post(res.results, inputs)
    if res.exec_time_ns is not None:
        print(f"HW exec time: {res.exec_time_ns} ns")
    return out


# revision 24
# speedup vs baseline: 36.3537x; 36.3537x over previous
"""Trainium2 Bass kernel for nn_FPModule (knn-interpolate + 2-layer MLP).

Sharding (per hint): 16 clouds over 8 NeuronCores, 2 clouds/core; kNN,
interpolation and MLP fully local per cloud; MLP weights replicated.

Per-core pipeline, software-pipelined at 8-chunk-group granularity so PE,
DVE, GPSIMD(SWDGE) and ACT overlap across groups:
 1. PE: s = -d2[fine, coarse] via a K=13 augmented fp16 hi/lo-split matmul
    on centroid-shifted coords (|err| ~3e-7 vs exact).
 2. DVE: max8 + max_index per 128-row chunk -> approx top-4 candidate
    coarse indices per fine point.
 3. GPSIMD indirect DMA: gather each candidate's 528B row (256 fp16 feats
    + raw fp32 position embedded in the row tail).
 4. DVE: recompute candidate d2 exactly with the reference formula
    (fp32 (q-p)^2 sums from the gathered positions), re-rank, and build
    exact inverse-distance weights (fixes near-tie neighbor flips that a
    matmul-expansion d2 alone cannot avoid).
 5. PE: fused weighted-sum + transpose: y.T accumulates in PSUM as
    sum_k feat_k.T @ diag(w_k), diag built by one DVE tensor_scalar.
 6. PE: MLP in transposed orientation (weights stationary, fine moving),
    ACT applies bias+relu; out.T streamed to HBM per 512-column tile,
    interleaved with the main pipeline.
 7. Host reassembles [65536, 256] fp32 (+ passthrough pos_skip/batch_skip).

Numerics vs fp32 reference: scale-relative absmax ~4e-4, rel-L2 ~5e-4.
Cost-model (TimelineSim) estimate: ~335 us/core; bottleneck is SWDGE
descriptor generation for the 32768 gathered rows (~266 us), with the
DVE scan (~222 us) hidden under it.
"""

import os
import sys
import numpy as np

sys.path.insert(0, "/opt/trn_rl_repo")

import concourse.bass as bass
import concourse.bacc as bacc
import concourse.mybir as mybir
from concourse.tile import TileContext
from concourse import bass_utils

FP16 = mybir.dt.float16
FP32 = mybir.dt.float32
U16 = mybir.dt.uint16
U32 = mybir.dt.uint32
I16 = mybir.dt.int16

B, NCO, NF = 16, 1024, 4096
CIN, CSK, CMID, COUT = 256, 128, 256, 256
CORES, CPC = 8, 2            # cores, clouds per core
CH = NF // 128               # 32 chunks of 128 fine points per cloud
KAUG = 13                    # augmented contraction rows for s = -d2
K = 3
J = 4                        # re-rank candidate count
ROWE = CIN + 8               # xaug row elements (256 feat + 12B fp32 pos + pad)

TENSOR_SPECS = [
    # name, shape, dtype, kind
    ("qa",   [CPC, KAUG, NF],       FP16, "ExternalInput"),
    ("pa",   [CPC, KAUG, NCO],      FP16, "ExternalInput"),
    ("xg",   [CPC * NCO, CIN + 8],  FP16, "ExternalInput"),
    ("qpos", [CPC, 128, 512],       FP32, "ExternalInput"),
    ("xst",  [CPC, CSK, NF],        FP16, "ExternalInput"),
    ("w1",   [128, 3 * 2 * 128],    FP16, "ExternalInput"),
    ("w2",   [128, 2 * 2 * 128],    FP16, "ExternalInput"),
    ("b1",   [128, 2],              FP32, "ExternalInput"),
    ("b2",   [128, 2],              FP32, "ExternalInput"),
    ("ident", [128, 128],           FP16, "ExternalInput"),
    ("outT", [CPC, 2, 128, NF],     FP32, "ExternalOutput"),
]


def build_kernel(tc, outs, ins):
    """Emit the per-core program. outs/ins: dicts name -> bass.AP."""
    from contextlib import ExitStack
    nc = tc.nc
    io = dict(ins)
    io.update(outs)

    with ExitStack() as ctx:
        const = ctx.enter_context(tc.tile_pool(name="const", bufs=1))
        spool = ctx.enter_context(tc.tile_pool(name="spsum", bufs=2, space="PSUM"))
        ypsum = ctx.enter_context(tc.tile_pool(name="ypsum", bufs=1, space="PSUM"))
        mpsum = ctx.enter_context(tc.tile_pool(name="mpsum", bufs=2, space="PSUM"))
        scan = ctx.enter_context(tc.tile_pool(name="scan", bufs=1))
        gpool = ctx.enter_context(tc.tile_pool(name="gath", bufs=3))
        ypool = ctx.enter_context(tc.tile_pool(name="ymac", bufs=3))
        tpool = ctx.enter_context(tc.tile_pool(name="tmp", bufs=3))
        ytpool = ctx.enter_context(tc.tile_pool(name="ytp", bufs=1))
        hpool = ctx.enter_context(tc.tile_pool(name="hp", bufs=1))
        opool = ctx.enter_context(tc.tile_pool(name="op", bufs=4))

        # ---- constant loads (per core) ----
        def load_const(name, shape, dtype, src_ap, tag):
            t = const.tile(shape, dtype, tag=tag)
            nc.sync.dma_start(t[:], src_ap)
            return t

        qa_sb, pa_sb, xst_sb = [], [], []
        for cl in range(CPC):
            qa_sb.append(load_const("qa", [KAUG, NF], FP16, io["qa"][cl], f"qa{cl}"))
            pa_sb.append(load_const("pa", [KAUG, NCO], FP16, io["pa"][cl], f"pa{cl}"))
            xst_sb.append(load_const("xst", [CSK, NF], FP16, io["xst"][cl], f"xst{cl}"))
        w1_sb = load_const("w1", [128, 3 * 2 * 128], FP16, io["w1"], "w1")
        w2_sb = load_const("w2", [128, 2 * 2 * 128], FP16, io["w2"], "w2")
        b1_sb = load_const("b1", [128, 2], FP32, io["b1"], "b1")
        b2_sb = load_const("b2", [128, 2], FP32, io["b2"], "b2")
        id_sb = load_const("ident", [128, 128], FP16, io["ident"], "ident")

        def w1t(k, mt):
            return w1_sb[:, (k * 2 + mt) * 128:(k * 2 + mt + 1) * 128]

        def w2t(k, mt):
            return w2_sb[:, (k * 2 + mt) * 128:(k * 2 + mt + 1) * 128]

        qp_sb, yts, state = [], [], {}
        for cl in range(CPC):
            qp = scan.tile([128, 512], FP32, tag=f"qp{cl}", name=f"qp{cl}")
            nc.sync.dma_start(qp[:], io["qpos"][cl])
            qp_sb.append(qp)
            yts.append([ytpool.tile([128, NF], FP16, tag=f"yt{cl}{mt}",
                                    name=f"yt{cl}_{mt}") for mt in range(2)])
        xg_flat = io["xg"]
        jobs = [(cl, grp) for cl in range(CPC) for grp in range(4)]

        def emit_scan(cl, grp):
            VALS = scan.tile([128, 64], FP32, tag=f"vals{cl}{grp}", name="VALS")
            IDX = scan.tile([128, 64], U32, tag=f"idx{cl}{grp}", name="IDX")
            for ci in range(8):
                c = grp * 8 + ci
                sp = spool.tile([128, NCO], FP32, tag="s", name="sp")
                for h in range(2):
                    nc.tensor.matmul(
                        sp[:, h * 512:(h + 1) * 512],
                        lhsT=qa_sb[cl][:, c * 128:(c + 1) * 128],
                        rhs=pa_sb[cl][:, h * 512:(h + 1) * 512],
                        start=True, stop=True)
                nc.vector.max(VALS[:, ci * 8:(ci + 1) * 8], sp[:])
                nc.vector.max_index(IDX[:, ci * 8:(ci + 1) * 8],
                                    VALS[:, ci * 8:(ci + 1) * 8], sp[:])
            state[(cl, grp, "IDX")] = IDX

        def emit_gathers(cl, grp):
            IDX = state[(cl, grp, "IDX")]
            gts = []
            for ci in range(8):
                G = gpool.tile([128, J * ROWE], FP16, tag="G", name="G", bufs=10)
                for k in range(J):
                    nc.gpsimd.indirect_dma_start(
                        out=G[:, k * ROWE:(k + 1) * ROWE], out_offset=None,
                        in_=xg_flat,
                        in_offset=bass.IndirectOffsetOnAxis(
                            ap=IDX[:, ci * 8 + k:ci * 8 + k + 1], axis=0),
                        element_offset=cl * NCO * ROWE)
                gts.append(G)
            state[(cl, grp, "gts")] = gts

        def emit_rank(cl, grp):
            gts = state[(cl, grp, "gts")]
            D2N = scan.tile([128, 64], FP32, tag=f"d2n{cl}{grp}", name="D2N")
            M8 = scan.tile([128, 64], FP32, tag=f"m8{cl}{grp}", name="M8")
            WT = scan.tile([128, 8 * J], FP32, tag=f"wt{cl}{grp}", name="WT")
            SEL = scan.tile([128, 8 * J], FP32, tag=f"sel{cl}{grp}", name="SEL")
            RC = scan.tile([128, 8 * J], FP32, tag=f"rc{cl}{grp}", name="RC")
            WS = scan.tile([128, 8], FP32, tag=f"ws{cl}{grp}", name="WS")
            RS = scan.tile([128, 8], FP32, tag=f"rs{cl}{grp}", name="RS")
            d2n3 = D2N[:].rearrange("p (c j) -> p c j", j=8)
            nc.vector.memset(d2n3[:, :, J:8], -1e30)
            for ci in range(8):
                c = grp * 8 + ci
                G = gts[ci]
                gv = G[:].rearrange("p (k e) -> p k e", e=ROWE)
                posv = gv[:, :, CIN:CIN + 6].bitcast(FP32)  # [128, J, 3]
                qp3 = qp_sb[cl][:, c * 16:(c + 1) * 16].rearrange(
                    "p (k d) -> p k d", d=4)[:, :, 0:3]
                dif = tpool.tile([128, J * 3], FP32, tag="dif", name="dif")
                d3v = dif[:].rearrange("p (k d) -> p k d", d=3)
                nc.vector.tensor_tensor(d3v, posv, qp3,
                                        op=mybir.AluOpType.subtract)
                sq = tpool.tile([128, J * 3], FP32, tag="sq", name="sq")
                s3v = sq[:].rearrange("p (k d) -> p k d", d=3)
                nc.vector.tensor_tensor(s3v, d3v, d3v, op=mybir.AluOpType.mult)
                nc.vector.tensor_reduce(d2n3[:, ci, 0:J], s3v,
                                        axis=mybir.AxisListType.X,
                                        op=mybir.AluOpType.add, negate=True)
                nc.vector.max(M8[:, ci * 8:(ci + 1) * 8],
                              D2N[:, ci * 8:(ci + 1) * 8])
            # batched exact weights (reference formula, fp32)
            m83 = M8[:].rearrange("p (c j) -> p c j", j=8)
            dn = d2n3[:, :, 0:J]
            sel3 = SEL[:].rearrange("p (c j) -> p c j", j=J)
            nc.vector.tensor_tensor(sel3, dn,
                                    m83[:, :, 2:3].to_broadcast([128, 8, J]),
                                    op=mybir.AluOpType.is_ge)
            rc3 = RC[:].rearrange("p (c j) -> p c j", j=J)
            nc.vector.tensor_scalar(rc3, dn, -1.0, 1e-16,
                                    op0=mybir.AluOpType.mult,
                                    op1=mybir.AluOpType.max)
            nc.vector.reciprocal(RC[:], RC[:])
            nc.vector.tensor_mul(RC[:], RC[:], SEL[:])
            nc.vector.tensor_reduce(WS[:], rc3, axis=mybir.AxisListType.X,
                                    op=mybir.AluOpType.add)
            nc.vector.reciprocal(RS[:], WS[:])
            wt3 = WT[:].rearrange("p (c j) -> p c j", j=J)
            rs3 = RS[:].rearrange("p (c o) -> p c o", o=1)
            nc.vector.tensor_tensor(wt3, rc3, rs3.to_broadcast([128, 8, J]),
                                    op=mybir.AluOpType.mult)
            state[(cl, grp, "WT")] = WT

        def emit_mac(cl, grp):
            gts = state[(cl, grp, "gts")]
            WT = state[(cl, grp, "WT")]
            yt = yts[cl]
            ypq = state.setdefault((cl, "ypq"), [None, None])
            for ci in range(8):
                c = grp * 8 + ci
                G = gts[ci]
                dgs = []
                for k in range(J):
                    dg = tpool.tile([128, 128], FP16, tag=f"dg{k}", name=f"dg{k}")
                    nc.vector.tensor_scalar(dg[:], id_sb[:],
                                            WT[:, ci * J + k:ci * J + k + 1],
                                            None, op0=mybir.AluOpType.mult)
                    dgs.append(dg)
                q = c % 4
                for mt in range(2):
                    if q == 0:
                        ypq[mt] = ypsum.tile([128, 512], FP32, tag=f"yp{mt}",
                                             name=f"yp{mt}")
                    for k in range(J):
                        nc.tensor.matmul(
                            ypq[mt][:, q * 128:(q + 1) * 128],
                            lhsT=G[:, k * ROWE + mt * 128:k * ROWE + (mt + 1) * 128],
                            rhs=dgs[k][:],
                            start=(k == 0), stop=(k == J - 1))
                if q == 3:
                    qq = c // 4
                    for mt in range(2):
                        nc.vector.tensor_copy(yt[mt][:, qq * 512:(qq + 1) * 512],
                                              ypq[mt][:])

        def emit_mlp_nt(cl, nt):
            yt = yts[cl]
            key = (cl, "ht")
            if key not in state:
                state[key] = [hpool.tile([128, NF], FP16, tag=f"ht{cl}{mt}",
                                         name=f"ht{cl}_{mt}") for mt in range(2)]
            ht = state[key]
            rhs1 = [yt[0], yt[1], xst_sb[cl]]
            sl = slice(nt * 512, (nt + 1) * 512)
            for mt in range(2):
                mp = mpsum.tile([128, 512], FP32, tag="mp", name="mp")
                for k in range(3):
                    nc.tensor.matmul(mp[:], lhsT=w1t(k, mt), rhs=rhs1[k][:, sl],
                                     start=(k == 0), stop=(k == 2))
                nc.scalar.activation(ht[mt][:, sl], mp[:],
                                     mybir.ActivationFunctionType.Relu,
                                     bias=b1_sb[:, mt:mt + 1], scale=1.0)
            for mt in range(2):
                mp = mpsum.tile([128, 512], FP32, tag="mp", name="mp")
                for k in range(2):
                    nc.tensor.matmul(mp[:], lhsT=w2t(k, mt), rhs=ht[k][:, sl],
                                     start=(k == 0), stop=(k == 1))
                o = opool.tile([128, 512], FP32, tag="o", name="o")
                nc.scalar.activation(o[:], mp[:],
                                     mybir.ActivationFunctionType.Relu,
                                     bias=b2_sb[:, mt:mt + 1], scale=1.0)
                nc.sync.dma_start(io["outT"][cl, mt][:, sl], o[:])

        # software-pipelined emission: keep Pool (gathers) fed while DVE
        # scans the next group; MLP per cloud after its last MAC.
        emit_scan(*jobs[0])
        for j, (cl, grp) in enumerate(jobs):
            emit_gathers(cl, grp)
            if j + 1 < len(jobs):
                emit_scan(*jobs[j + 1])
            emit_rank(cl, grp)
            emit_mac(cl, grp)
            emit_mlp_nt(cl, 2 * grp)
            emit_mlp_nt(cl, 2 * grp + 1)


def host_prep(inputs):
    """Full inputs -> list of 8 per-core in_maps (numpy)."""
    x = np.asarray(inputs["x"], np.float32)
    pos = np.asarray(inputs["pos"], np.float64)
    x_skip = np.asarray(inputs["x_skip"], np.float32)
    pos_skip = np.asarray(inputs["pos_skip"], np.float64)
    W1 = np.asarray(inputs["W1"], np.float32)
    b1 = np.asarray(inputs["b1"], np.float32)
    W2 = np.asarray(inputs["W2"], np.float32)
    b2 = np.asarray(inputs["b2"], np.float32)

    ident = np.eye(128, dtype=np.float16)
    # w[p, (k*2+mt)*128+f] = W[k*128+p, mt*128+f]
    w1r = np.ascontiguousarray(
        W1.astype(np.float16).reshape(3, 128, 2, 128)
        .transpose(1, 0, 2, 3).reshape(128, 3 * 2 * 128))
    w2r = np.ascontiguousarray(
        W2.astype(np.float16).reshape(2, 128, 2, 128)
        .transpose(1, 0, 2, 3).reshape(128, 2 * 2 * 128))
    b1r = np.ascontiguousarray(b1.reshape(2, 128).T)
    b2r = np.ascontiguousarray(b2.reshape(2, 128).T)

    in_maps = []
    for core in range(CORES):
        qa = np.zeros((CPC, KAUG, NF), np.float16)
        pa = np.zeros((CPC, KAUG, NCO), np.float16)
        xg = np.zeros((CPC * NCO, CIN + 8), np.float16)
        qpos = np.zeros((CPC, 128, 512), np.float32)
        xst = np.zeros((CPC, CSK, NF), np.float16)
        for cl in range(CPC):
            cloud = core * CPC + cl
            pf = pos[cloud * NCO:(cloud + 1) * NCO]
            qf = pos_skip[cloud * NF:(cloud + 1) * NF]
            xf = x[cloud * NCO:(cloud + 1) * NCO]
            xsf = x_skip[cloud * NF:(cloud + 1) * NF]

            mu = pf.mean(0)
            qp = qf - mu
            pp = pf - mu
            qh = qp.astype(np.float16)
            ql = (qp - qh.astype(np.float64)).astype(np.float16)
            ph = pp.astype(np.float16)
            pl = (pp - ph.astype(np.float64)).astype(np.float16)
            qs = qh.astype(np.float64) + ql.astype(np.float64)
            ps = ph.astype(np.float64) + pl.astype(np.float64)
            nq2 = -(qs * qs).sum(1)
            np2 = -(ps * ps).sum(1)
            nq2h = nq2.astype(np.float16)
            nq2l = (nq2 - nq2h.astype(np.float64)).astype(np.float16)
            np2h = np2.astype(np.float16)
            np2l = (np2 - np2h.astype(np.float64)).astype(np.float16)
            one_f = np.ones(NF, np.float16)
            one_c = np.ones(NCO, np.float16)
            qa[cl] = np.stack([
                2 * qh[:, 0], 2 * qh[:, 1], 2 * qh[:, 2],
                2 * qh[:, 0], 2 * qh[:, 1], 2 * qh[:, 2],
                2 * ql[:, 0], 2 * ql[:, 1], 2 * ql[:, 2],
                nq2h, nq2l, one_f, one_f])
            pa[cl] = np.stack([
                ph[:, 0], ph[:, 1], ph[:, 2],
                pl[:, 0], pl[:, 1], pl[:, 2],
                ph[:, 0], ph[:, 1], ph[:, 2],
                one_c, one_c, np2h, np2l])
            row = xg[cl * NCO:(cl + 1) * NCO]
            row[:, :CIN] = xf.astype(np.float16)
            # embed raw fp32 coarse positions in the row tail (bytes 512..524)
            row[:, CIN:CIN + 6] = pf.astype(np.float32).view(np.float16)
            # qpos[cl, p, c*16 + k*4 + d] = fine-point coord (fp32, replicated J times)
            qf32 = qf.astype(np.float32).reshape(CH, 128, 3)
            qp4 = np.zeros((CH, 128, J, 4), np.float32)
            qp4[:, :, :, 0:3] = qf32[:, :, None, :]
            qpos[cl] = qp4.transpose(1, 0, 2, 3).reshape(128, CH * 16)
            xst[cl] = np.ascontiguousarray(xsf.astype(np.float16).T)
        in_maps.append({
            "qa": qa, "pa": pa, "xg": xg, "qpos": qpos, "xst": xst,
            "w1": w1r, "w2": w2r, "b1": b1r, "b2": b2r, "ident": ident,
        })
    return in_maps


def host_post(results, inputs):
    out = np.empty((B * NF, COUT), np.float32)
    for core in range(CORES):
        o = results[core]["outT"]  # [CPC, 2, 128, NF]
        for cl in range(CPC):
            cloud = core * CPC + cl
            blk = o[cl].reshape(COUT, NF)  # [256, 4096]
            out[cloud * NF:(cloud + 1) * NF] = blk.T
    pos_skip = np.asarray(inputs["pos_skip"])
    batch_skip = np.asarray(inputs["batch_skip"])
    return (out, pos_skip, batch_skip)


def make_program():
    nc = bacc.Bacc("TRN2", target_bir_lowering=False, debug=False,
                   enable_asserts=False, num_devices=CORES)
    ins, outs = {}, {}
    for name, shape, dt, kind in TENSOR_SPECS:
        ap = nc.dram_tensor(name, shape, dt, kind=kind).ap()
        (outs if kind == "ExternalOutput" else ins)[name] = ap
    with TileContext(nc) as tc:
        build_kernel(tc, outs, ins)
    nc.compile()
    return nc


_PROGRAM_CACHE = {}


def kernel(**inputs):
    in_maps = host_prep(inputs)
    if "nc" not in _PROGRAM_CACHE:
        _PROGRAM_CACHE["nc"] = make_program()
    nc = _PROGRAM_CACHE["nc"]
    res = bass_utils.run_bass_kernel_spmd(
        nc, in_maps, core_ids=list(range(CORES)),
        trace=bool(int(os.environ.get("KERNEL_TRACE", "0"))),
    )
    out = host_post(res.results, inputs)
    if res.exec_time_ns is not None:
        print(f"HW exec time: {res.exec_time_ns} ns")
    return out
